# revision 10
# baseline (speedup 1.0000x reference)
"""Distributed paged GQA decode attention for Trainium2 (8 NeuronCores).

Strategy
--------
The 256 independent (batch, kv_head) pairs are the unit of work.  For each
pair the output depends only on the first seq_len+1 tokens of its paged
context, so the host gathers exactly the valid tokens from the paged cache
(emulating the decode_store_kv scatter first), pre-transposes K, folds the
softmax scale into q, casts everything to bf16, and ships per-core blobs.

Pairs are sorted by length and dealt into 32 groups of 8; group j becomes
"slot j" on every core (core c takes rank c of group j).  All cores share
one chunk count C_j = ceil(maxlen(group j)/128), which keeps the single
SPMD instruction stream identical across cores while wasting only ~10% in
padding.

Per core the device kernel holds everything in SBUF and, per slot j:
  scoresT[l,g] = KT_chunk.T @ qT          (PE, chunked by 128 tokens)
  e = exp(scoresT)                        (ACT, psum -> sbuf, bf16)
  o_unnorm/denom = e.T @ [V | 1]          (PE, accumulated in psum)
softmax max-subtraction is unnecessary (|score| <= ~7 for this regime) and
cancels between numerator and denominator; padded tokens contribute zero
because their V rows AND the ones-column are zeroed.  The final division
happens on the host during the unshard.
"""

import sys

sys.path.insert(0, "/opt/trn_rl_repo")

import numpy as np
import ml_dtypes

B = 32
H = 32
HKV = 8
D = 128
P = 16
G = H // HKV          # 4 query heads per kv head
SCALE = 0.08838834764831845
N_CORES = 8
CHUNK = 128
N_SLOTS = (B * HKV) // N_CORES   # 32 slots per core

BF16 = ml_dtypes.bfloat16

_GRAPH_CACHE = {}


QTW = N_SLOTS * G


def _layout(C):
    """kv blob column layout (bf16).

    group 0: [qt QTW cols | kt slots | vd slots]; group g: [kt | vd].
    <=7 input DMAs + 4 output DMAs keeps every DMA on its own semaphore
    lane (8 HWDGE + 8 SWDGE), avoiding lane-recycle stalls that pace the
    input stream to compute speed.
    """
    sizes = [2, 4, 4, 8, 8, 4, 2]
    assert sum(sizes) == N_SLOTS
    groups = []
    pos = 0
    for s in sizes:
        groups.append(list(range(pos, pos + s)))
        pos += s
    kt_off = {}
    vd_off = {}
    grp_off = []          # (blob col offset, width) per group
    w = 0
    for gi, slots in enumerate(groups):
        base = w
        cur = base + (QTW if gi == 0 else 0)
        for j in slots:
            kt_off[j] = cur
            cur += C[j] * CHUNK
        for j in slots:
            vd_off[j] = cur
            cur += C[j] * 129
        grp_off.append((base, cur - base))
        w = cur
    return groups, kt_off, vd_off, grp_off, w


def _build_graph(C):
    """Build the SPMD Bass graph for per-slot chunk counts C (len 32)."""
    from concourse import bacc, tile, mybir, bass

    NCH = sum(C)
    T = NCH * CHUNK
    groups, kt_off, vd_off, grp_off, WTOT = _layout(C)
    assert WTOT == QTW + T + NCH * 129

    nc = bacc.Bacc("TRN2", target_bir_lowering=False, debug=False,
                   num_devices=N_CORES)
    kv_d = nc.dram_tensor("kv", [128, WTOT], mybir.dt.bfloat16,
                          kind="ExternalInput")
    out_d = nc.dram_tensor("out", [G, N_SLOTS * 129], mybir.dt.float32,
                           kind="ExternalOutput")

    with tile.TileContext(nc) as tc:
        with (
            tc.tile_pool(name="data", bufs=1) as data_pool,
            tc.tile_pool(name="work", bufs=1) as work_pool,
            tc.tile_pool(name="psum", bufs=1, space=bass.MemorySpace.PSUM)
                as psum_pool,
        ):
            grp_tiles = []
            for gi, slots in enumerate(groups):
                base, width = grp_off[gi]
                kv_g = data_pool.tile([128, width], mybir.dt.bfloat16,
                                      tag=f"kv{gi}", name=f"kv{gi}")
                nc.sync.dma_start(out=kv_g[:],
                                  in_=kv_d.ap()[:, base:base + width])
                grp_tiles.append(kv_g)
            grp_of_slot = {}
            for gi, slots in enumerate(groups):
                for j in slots:
                    grp_of_slot[j] = gi

            def sl(j, off, width):
                gi = grp_of_slot[j]
                base, _ = grp_off[gi]
                return grp_tiles[gi][:, off - base:off - base + width]

            qt_tile = grp_tiles[0]          # qt at cols [0, QTW)

            # output staging: 4 tiles of 8 slots each; one gpsimd (SWDGE)
            # DMA per stage so the HWDGE input ring never stalls on the
            # output's HBM-write receipt
            OUT_GRP = 8
            stage_tiles = [
                work_pool.tile([G, OUT_GRP * 129], mybir.dt.float32,
                               tag=f"stage{s}", name=f"stage{s}")
                for s in range(N_SLOTS // OUT_GRP)
            ]

            def emit_pv(j, e):
                po = psum_pool.tile([G, 129], mybir.dt.float32,
                                    tag="po", bufs=3, name=f"po{j}")
                for c in range(C[j]):
                    nc.tensor.matmul(
                        po[:, :],
                        e[:, G * c:G * (c + 1)],
                        sl(j, vd_off[j] + c * 129, 129),
                        start=(c == 0), stop=(c == C[j] - 1),
                    )
                s, r = divmod(j, OUT_GRP)
                nc.vector.tensor_copy(
                    stage_tiles[s][:, r * 129:(r + 1) * 129], po[:])
                if r == OUT_GRP - 1:
                    nc.gpsimd.dma_start(
                        out=out_d.ap()[:, s * OUT_GRP * 129:
                                       (s + 1) * OUT_GRP * 129],
                        in_=stage_tiles[s][:],
                    )

            prev = None
            for j in range(N_SLOTS):
                scores = psum_pool.tile([128, G * C[j]], mybir.dt.float32,
                                        tag="scores", bufs=3,
                                        name=f"scores{j}")
                for c in range(C[j]):
                    nc.tensor.matmul(
                        scores[:, G * c:G * (c + 1)],
                        sl(j, kt_off[j] + c * CHUNK, CHUNK),
                        qt_tile[:, G * j:G * (j + 1)],
                        start=True, stop=True,
                    )
                e = work_pool.tile([128, G * C[j]], mybir.dt.bfloat16,
                                   tag="e", bufs=3, name=f"e{j}")
                nc.scalar.activation(e[:], scores[:],
                                     mybir.ActivationFunctionType.Exp)
                if prev is not None:
                    emit_pv(*prev)
                prev = (j, e)
            emit_pv(*prev)

    nc.compile()
    return nc


def _prepare(q, k, v, k_cache, v_cache, bh_seq_lens, page_table,
             batch_mapping):
    """Host-side shard planning + gather.  Returns (C, in_maps, pair_map)."""
    q = np.asarray(q, dtype=np.float32)
    k = np.asarray(k, dtype=np.float32)
    v = np.asarray(v, dtype=np.float32)
    kcf = np.asarray(k_cache, dtype=np.float32).reshape(-1, D).copy()
    vcf = np.asarray(v_cache, dtype=np.float32).reshape(-1, D).copy()
    sl = np.asarray(bh_seq_lens)
    pt = np.asarray(page_table)
    bm = np.asarray(batch_mapping)

    seq = sl[bm]                      # [B, HKV]
    ptb = pt[bm].astype(np.int64)     # [B, HKV, M]

    # decode_store_kv: scatter new token into cache copies
    page_of = np.take_along_axis(ptb, (seq // P)[..., None].astype(np.int64),
                                 axis=-1)[..., 0]
    flat = page_of * P + (seq % P)
    kcf[flat.reshape(-1)] = k.reshape(-1, D)
    vcf[flat.reshape(-1)] = v.reshape(-1, D)

    lens = (seq + 1).reshape(-1)               # [256] valid tokens per pair
    order = np.argsort(-lens, kind="stable")   # longest first
    # group j = pairs order[8j..8j+8); core c <- rank c
    C = []
    for j in range(N_SLOTS):
        grp = order[N_CORES * j:N_CORES * (j + 1)]
        C.append(int(np.ceil(lens[grp].max() / CHUNK)))
    _, kt_off, vd_off, _, WTOT = _layout(C)

    in_maps = []
    pair_map = []  # per core: list of (b, h) per slot
    for c in range(N_CORES):
        KV = np.zeros((128, WTOT), dtype=BF16)
        pm = []
        for j in range(N_SLOTS):
            pair = int(order[N_CORES * j + c])
            b, h = pair // HKV, pair % HKV
            pm.append((b, h))
            L = int(lens[pair])
            npages = (L + P - 1) // P
            tok = (ptb[b, h, :npages, None] * P
                   + np.arange(P, dtype=np.int64)).reshape(-1)[:L]
            KV[:, kt_off[j]:kt_off[j] + L] = kcf[tok].T.astype(BF16)
            V3 = np.zeros((C[j] * 128, 129), dtype=BF16)
            V3[:L, :D] = vcf[tok].astype(BF16)
            V3[:L, D] = np.float32(1.0)
            KV[:, vd_off[j]:vd_off[j] + C[j] * 129] = (
                V3.reshape(C[j], 128, 129).transpose(1, 0, 2)
                .reshape(128, C[j] * 129))
            KV[:, G * j:G * (j + 1)] = \
                (q[b, h * G:(h + 1) * G] * SCALE).T.astype(BF16)
        in_maps.append({"kv": KV})
        pair_map.append(pm)
    return tuple(C), in_maps, pair_map


def _run(inputs, trace=False, trace_cores=None):
    from concourse.bass_utils import run_bass_kernel_spmd

    C, in_maps, pair_map = _prepare(**inputs)
    if C not in _GRAPH_CACHE:
        _GRAPH_CACHE[C] = _build_graph(list(C))
    nc = _GRAPH_CACHE[C]

    res = run_bass_kernel_spmd(
        nc, in_maps, core_ids=list(range(N_CORES)),
        trace=trace, trace_cores=trace_cores,
    )

    out = np.zeros((B, H, D), dtype=np.float32)
    for c in range(N_CORES):
        oc = np.asarray(res.results[c]["out"], dtype=np.float32)
        oc = oc.reshape(G, N_SLOTS, 129).transpose(1, 0, 2)  # [slot, g, 129]
        for j, (b, h) in enumerate(pair_map[c]):
            out[b, h * G:(h + 1) * G] = oc[j, :, :D] / oc[j, :, D:D + 1]
    return out, res


def kernel(q, k, v, k_cache, v_cache, bh_seq_lens, page_table,
           batch_mapping):
    out, _ = _run(dict(q=q, k=k, v=v, k_cache=k_cache, v_cache=v_cache,
                       bh_seq_lens=bh_seq_lens, page_table=page_table,
                       batch_mapping=batch_mapping))
    return out


# revision 11
# speedup vs baseline: 1.0615x; 1.0615x over previous
"""Distributed paged GQA decode attention for Trainium2 (8 NeuronCores).

Strategy
--------
The 256 independent (batch, kv_head) pairs are the unit of work.  For each
pair the output depends only on the first seq_len+1 tokens of its paged
context, so the host gathers exactly the valid tokens from the paged cache
(emulating the decode_store_kv scatter first), pre-transposes K, folds the
softmax scale into q, casts everything to bf16, and ships per-core blobs.

Pairs are sorted by length and dealt into 32 groups of 8; group j becomes
"slot j" on every core (core c takes rank c of group j).  All cores share
one chunk count C_j = ceil(maxlen(group j)/128), which keeps the single
SPMD instruction stream identical across cores while wasting only ~10% in
padding.

Per core the device kernel holds everything in SBUF and, per slot j:
  scoresT[l,g] = KT_chunk.T @ qT          (PE, chunked by 128 tokens)
  e = exp(scoresT)                        (ACT, psum -> sbuf, bf16)
  o_unnorm/denom = e.T @ [V | 1]          (PE, accumulated in psum)
softmax max-subtraction is unnecessary (|score| <= ~7 for this regime) and
cancels between numerator and denominator; padded tokens contribute zero
because their V rows AND the ones-column are zeroed.  The final division
happens on the host during the unshard.
"""

import sys

sys.path.insert(0, "/opt/trn_rl_repo")

import numpy as np
import ml_dtypes

B = 32
H = 32
HKV = 8
D = 128
P = 16
G = H // HKV          # 4 query heads per kv head
SCALE = 0.08838834764831845
N_CORES = 8
CHUNK = 128
N_SLOTS = (B * HKV) // N_CORES   # 32 slots per core

BF16 = ml_dtypes.bfloat16

_GRAPH_CACHE = {}


QTW = N_SLOTS * G


def _layout(C):
    """kv blob column layout (bf16).

    group 0: [qt QTW cols | kt slots | vd slots]; group g: [kt | vd].
    <=7 input DMAs + 4 output DMAs keeps every DMA on its own semaphore
    lane (8 HWDGE + 8 SWDGE), avoiding lane-recycle stalls that pace the
    input stream to compute speed.
    """
    # greedy grouping by chunk budget: small first group for an early
    # compute start, then ~uniform ~1.3MB transfers
    groups = []
    cur = []
    budget = 8
    acc = 0
    for j in range(N_SLOTS):
        cur.append(j)
        acc += C[j]
        if acc >= budget:
            groups.append(cur)
            cur = []
            acc = 0
            budget = 20
    if cur:
        groups.append(cur)
    kt_off = {}
    vd_off = {}
    grp_off = []          # (blob col offset, width) per group
    w = 0
    for gi, slots in enumerate(groups):
        base = w
        cur = base + (QTW if gi == 0 else 0)
        for j in slots:
            kt_off[j] = cur
            cur += C[j] * CHUNK
        for j in slots:
            vd_off[j] = cur
            cur += C[j] * 129
        grp_off.append((base, cur - base))
        w = cur
    return groups, kt_off, vd_off, grp_off, w


def _build_graph(C):
    """Build the SPMD Bass graph for per-slot chunk counts C (len 32)."""
    from concourse import bacc, tile, mybir, bass

    NCH = sum(C)
    T = NCH * CHUNK
    groups, kt_off, vd_off, grp_off, WTOT = _layout(C)
    assert WTOT == QTW + T + NCH * 129

    nc = bacc.Bacc("TRN2", target_bir_lowering=False, debug=False,
                   num_devices=N_CORES)
    kv_d = nc.dram_tensor("kv", [128, WTOT], mybir.dt.bfloat16,
                          kind="ExternalInput")
    out_d = nc.dram_tensor("out", [G, N_SLOTS * 129], mybir.dt.float32,
                           kind="ExternalOutput")

    with tile.TileContext(nc) as tc:
        with (
            tc.tile_pool(name="data", bufs=1) as data_pool,
            tc.tile_pool(name="work", bufs=1) as work_pool,
            tc.tile_pool(name="psum", bufs=1, space=bass.MemorySpace.PSUM)
                as psum_pool,
        ):
            grp_tiles = []
            for gi, slots in enumerate(groups):
                base, width = grp_off[gi]
                kv_g = data_pool.tile([128, width], mybir.dt.bfloat16,
                                      tag=f"kv{gi}", name=f"kv{gi}")
                nc.sync.dma_start(out=kv_g[:],
                                  in_=kv_d.ap()[:, base:base + width])
                grp_tiles.append(kv_g)
            grp_of_slot = {}
            for gi, slots in enumerate(groups):
                for j in slots:
                    grp_of_slot[j] = gi

            def sl(j, off, width):
                gi = grp_of_slot[j]
                base, _ = grp_off[gi]
                return grp_tiles[gi][:, off - base:off - base + width]

            qt_tile = grp_tiles[0]          # qt at cols [0, QTW)

            # output staging: 4 tiles of 8 slots each; one gpsimd (SWDGE)
            # DMA per stage so the HWDGE input ring never stalls on the
            # output's HBM-write receipt
            OUT_GRP = 8
            stage_tiles = [
                work_pool.tile([G, OUT_GRP * 129], mybir.dt.float32,
                               tag=f"stage{s}", name=f"stage{s}")
                for s in range(N_SLOTS // OUT_GRP)
            ]

            def emit_pv(j, e):
                po = psum_pool.tile([G, 129], mybir.dt.float32,
                                    tag="po", bufs=3, name=f"po{j}")
                for c in range(C[j]):
                    nc.tensor.matmul(
                        po[:, :],
                        e[:, G * c:G * (c + 1)],
                        sl(j, vd_off[j] + c * 129, 129),
                        start=(c == 0), stop=(c == C[j] - 1),
                    )
                s, r = divmod(j, OUT_GRP)
                nc.vector.tensor_copy(
                    stage_tiles[s][:, r * 129:(r + 1) * 129], po[:])
                if r == OUT_GRP - 1:
                    nc.gpsimd.dma_start(
                        out=out_d.ap()[:, s * OUT_GRP * 129:
                                       (s + 1) * OUT_GRP * 129],
                        in_=stage_tiles[s][:],
                    )

            prev = None
            for j in range(N_SLOTS):
                scores = psum_pool.tile([128, G * C[j]], mybir.dt.float32,
                                        tag="scores", bufs=3,
                                        name=f"scores{j}")
                for c in range(C[j]):
                    nc.tensor.matmul(
                        scores[:, G * c:G * (c + 1)],
                        sl(j, kt_off[j] + c * CHUNK, CHUNK),
                        qt_tile[:, G * j:G * (j + 1)],
                        start=True, stop=True,
                    )
                e = work_pool.tile([128, G * C[j]], mybir.dt.bfloat16,
                                   tag="e", bufs=3, name=f"e{j}")
                nc.scalar.activation(e[:], scores[:],
                                     mybir.ActivationFunctionType.Exp)
                if prev is not None:
                    emit_pv(*prev)
                prev = (j, e)
            emit_pv(*prev)

    nc.compile()
    return nc


def _prepare(q, k, v, k_cache, v_cache, bh_seq_lens, page_table,
             batch_mapping):
    """Host-side shard planning + gather.  Returns (C, in_maps, pair_map)."""
    q = np.asarray(q, dtype=np.float32)
    k = np.asarray(k, dtype=np.float32)
    v = np.asarray(v, dtype=np.float32)
    kcf = np.asarray(k_cache, dtype=np.float32).reshape(-1, D).copy()
    vcf = np.asarray(v_cache, dtype=np.float32).reshape(-1, D).copy()
    sl = np.asarray(bh_seq_lens)
    pt = np.asarray(page_table)
    bm = np.asarray(batch_mapping)

    seq = sl[bm]                      # [B, HKV]
    ptb = pt[bm].astype(np.int64)     # [B, HKV, M]

    # decode_store_kv: scatter new token into cache copies
    page_of = np.take_along_axis(ptb, (seq // P)[..., None].astype(np.int64),
                                 axis=-1)[..., 0]
    flat = page_of * P + (seq % P)
    kcf[flat.reshape(-1)] = k.reshape(-1, D)
    vcf[flat.reshape(-1)] = v.reshape(-1, D)

    lens = (seq + 1).reshape(-1)               # [256] valid tokens per pair
    order = np.argsort(-lens, kind="stable")   # longest first
    # group j = pairs order[8j..8j+8); core c <- rank c
    C = []
    for j in range(N_SLOTS):
        grp = order[N_CORES * j:N_CORES * (j + 1)]
        C.append(int(np.ceil(lens[grp].max() / CHUNK)))
    _, kt_off, vd_off, _, WTOT = _layout(C)

    in_maps = []
    pair_map = []  # per core: list of (b, h) per slot
    for c in range(N_CORES):
        KV = np.zeros((128, WTOT), dtype=BF16)
        pm = []
        for j in range(N_SLOTS):
            pair = int(order[N_CORES * j + c])
            b, h = pair // HKV, pair % HKV
            pm.append((b, h))
            L = int(lens[pair])
            npages = (L + P - 1) // P
            tok = (ptb[b, h, :npages, None] * P
                   + np.arange(P, dtype=np.int64)).reshape(-1)[:L]
            KV[:, kt_off[j]:kt_off[j] + L] = kcf[tok].T.astype(BF16)
            V3 = np.zeros((C[j] * 128, 129), dtype=BF16)
            V3[:L, :D] = vcf[tok].astype(BF16)
            V3[:L, D] = np.float32(1.0)
            KV[:, vd_off[j]:vd_off[j] + C[j] * 129] = (
                V3.reshape(C[j], 128, 129).transpose(1, 0, 2)
                .reshape(128, C[j] * 129))
            KV[:, G * j:G * (j + 1)] = \
                (q[b, h * G:(h + 1) * G] * SCALE).T.astype(BF16)
        in_maps.append({"kv": KV})
        pair_map.append(pm)
    return tuple(C), in_maps, pair_map


def _run(inputs, trace=False, trace_cores=None):
    from concourse.bass_utils import run_bass_kernel_spmd

    C, in_maps, pair_map = _prepare(**inputs)
    if C not in _GRAPH_CACHE:
        _GRAPH_CACHE[C] = _build_graph(list(C))
    nc = _GRAPH_CACHE[C]

    res = run_bass_kernel_spmd(
        nc, in_maps, core_ids=list(range(N_CORES)),
        trace=trace, trace_cores=trace_cores,
    )

    out = np.zeros((B, H, D), dtype=np.float32)
    for c in range(N_CORES):
        oc = np.asarray(res.results[c]["out"], dtype=np.float32)
        oc = oc.reshape(G, N_SLOTS, 129).transpose(1, 0, 2)  # [slot, g, 129]
        for j, (b, h) in enumerate(pair_map[c]):
            out[b, h * G:(h + 1) * G] = oc[j, :, :D] / oc[j, :, D:D + 1]
    return out, res


def kernel(q, k, v, k_cache, v_cache, bh_seq_lens, page_table,
           batch_mapping):
    out, _ = _run(dict(q=q, k=k, v=v, k_cache=k_cache, v_cache=v_cache,
                       bh_seq_lens=bh_seq_lens, page_table=page_table,
                       batch_mapping=batch_mapping))
    return out
